# revision 1
# baseline (speedup 1.0000x reference)
"""Sparse cross-attention kernel for Trainium2 (8 NeuronCores).

Sharding: valid tokens (rows of the NxN attention) are sharded across the 8
cores -- each core holds 1024 queries and the full gathered key/value set
(8192 tokens), computes its energy rows + softmax + output rows.

Device layout trick: energy is computed TRANSPOSED (eT[key, query], keys on
partitions) so that
  * the exp for softmax is a single ScalarE pass PSUM->SBUF (it doubles as
    the PSUM-evacuation copy),
  * the attention matmul consumes exp(eT) directly as the moving operand with
    token-major yt tiles as stationary weights (no transposes anywhere),
  * a ones-column appended to yt makes the softmax denominator fall out of
    the same accumulation for free.
The value/output projections fold into a single tiny (Wz @ Wt) matrix that is
applied on the host, together with the global group-norm statistics (which
need all 8192 tokens), the scatter back into the dense map, and the residual.
"""

import sys

import numpy as np

sys.path.insert(0, "/opt/trn_rl_repo")

import concourse.bacc as bacc  # noqa: E402
import concourse.tile as tile  # noqa: E402
from concourse import bass_utils, mybir  # noqa: E402

# problem constants (hardcoded per contract)
B, CQ, CK, F, H, W = 2, 32, 16, 64, 128, 128
NV = 8192               # number of valid (mask > 0) tokens
NCORES = 8
QL = NV // NCORES       # queries per core
KB = 128                # key block (partition dim of eT tiles)
NKB = NV // KB          # 64 key blocks
CA = CK + 1             # yt channels + ones column
NQMM = QL // 512        # moving-dim chunks per matmul (fp32 max free 512)
EPS = 1e-5

FP32 = mybir.dt.float32
FP32R = mybir.dt.float32r
BF16 = mybir.dt.bfloat16

USE_FP32R = True        # big matmuls in fp32r (1 cyc/row vs 4 for fp32)
MMDT = FP32R if USE_FP32R else FP32  # dtype of tiles feeding the energy matmul
# the attention matmul runs in bf16: fp32r forbids dst partitions != 0, which
# would kill column packing; bf16 error only enters via attention weights
ATTDT = BF16
COLPACK = True          # 2-way column packing of the attention matmul

_PROG = None
LAST_RESULTS = None     # BassKernelResults of the last run (for test harness)


def _body(tc, d_xtT, d_ytT, d_ytaug, d_wpT, d_wgT, d_out, niters=1):
    nc = tc.nc
    from contextlib import ExitStack

    with ExitStack() as ctx:
        const = ctx.enter_context(tc.tile_pool(name="const", bufs=1))
        xpool = ctx.enter_context(tc.tile_pool(name="xp", bufs=8))
        epool = ctx.enter_context(tc.tile_pool(name="ep", bufs=3, space="PSUM"))
        apool = ctx.enter_context(tc.tile_pool(name="acc", bufs=1, space="PSUM"))
        for _it in range(niters):
            _iter(nc, const, xpool, epool, apool,
                  d_xtT, d_ytT, d_ytaug, d_wpT, d_wgT, d_out)


def _iter(nc, const, xpool, epool, apool,
          d_xtT, d_ytT, d_ytaug, d_wpT, d_wgT, d_out):
    if True:

        # ---- input DMAs ----
        wp_s = const.tile([CQ, F], FP32)
        nc.sync.dma_start(out=wp_s[:], in_=d_wpT[:])
        wg_s = const.tile([CK, F], FP32)
        nc.sync.dma_start(out=wg_s[:], in_=d_wgT[:])
        xtT_s = const.tile([CQ, QL], FP32)
        nc.sync.dma_start(out=xtT_s[:], in_=d_xtT[:])
        # chunk the key DMA so the g-projection (and thus the main loop) can
        # start as soon as the first slice lands
        ytT_s = const.tile([CK, NV], FP32)
        for c in range(16):
            nc.sync.dma_start(
                out=ytT_s[:, c * 512:(c + 1) * 512],
                in_=d_ytT[:, c * 512:(c + 1) * 512],
            )
        ytaug_s = const.tile([128, NKB * CA], ATTDT)
        nc.sync.dma_start(out=ytaug_s[:], in_=d_ytaug[:])

        # ---- query projection: pT[f, q] = Wp @ xtT ----
        p_ps = epool.tile([F, QL], FP32, tag="et")
        for i in range(NQMM):
            nc.tensor.matmul(
                out=p_ps[:, i * 512:(i + 1) * 512],
                lhsT=wp_s[:],
                rhs=xtT_s[:, i * 512:(i + 1) * 512],
                start=True, stop=True,
            )
        pT_s = const.tile([F, QL], MMDT)
        nc.vector.tensor_copy(out=pT_s[:], in_=p_ps[:])

        # ---- key projection: gT[f, k] = Wg @ ytT ----
        gT_s = const.tile([F, NV], MMDT)
        for c in range(16):
            g_ps = epool.tile([F, 512], FP32, tag="et")
            nc.tensor.matmul(
                out=g_ps[:],
                lhsT=wg_s[:],
                rhs=ytT_s[:, c * 512:(c + 1) * 512],
                start=True, stop=True,
            )
            nc.vector.tensor_copy(out=gT_s[:, c * 512:(c + 1) * 512], in_=g_ps[:])

        # ---- attention accumulator: [yt | 1].T @ exp(eT), [17, QL] per
        # column group. The two groups share PSUM banks at disjoint partition
        # ranges (0:17 and 64:81) -- the standard col-tiling layout; PSUM
        # has_written bits are per-element so the interleaved accumulation
        # groups don't interact (the sim's coarse zero-region tracker can't
        # see that, hence skip_group_check on the matmuls). ----
        outS_ps = apool.tile([128, QL], FP32)

        for j in range(NKB):
            # energy block (transposed): eT[k, q] = g_k . p_q
            e_ps = epool.tile([128, QL], FP32, tag="et")
            lhs_g = gT_s[:, j * KB:(j + 1) * KB]
            rhs_p = pT_s[:]
            for i in range(NQMM):
                nc.tensor.matmul(
                    out=e_ps[:, i * 512:(i + 1) * 512],
                    lhsT=lhs_g,
                    rhs=rhs_p[:, i * 512:(i + 1) * 512],
                    start=True, stop=True,
                )
            # softmax numerator: exp straight out of PSUM into SBUF.
            # No max subtraction: energies are O(+-50), well inside fp32 exp
            # range, and the reference's max-shift cancels mathematically.
            x_s = xpool.tile([128, QL], ATTDT, tag="xp")
            nc.scalar.activation(
                out=x_s[:], in_=e_ps[:], func=mybir.ActivationFunctionType.Exp
            )
            # attention matmul, accumulated over key blocks; even/odd blocks
            # go to different PE column groups (disjoint PSUM partitions) so
            # consecutive blocks run concurrently on the array
            grp = (j % 2) if COLPACK else 0
            bp = 64 * grp
            lhs_t = ytaug_s[:, j * CA:(j + 1) * CA]
            rhs_x = x_s[:]
            if COLPACK:
                start = j == grp
                stop = j == (NKB - 2 + grp)
            else:
                start = j == 0
                stop = j == NKB - 1
            for i in range(NQMM):
                nc.tensor.matmul(
                    out=outS_ps[bp:bp + CA, i * 512:(i + 1) * 512],
                    lhsT=lhs_t,
                    rhs=rhs_x[:, i * 512:(i + 1) * 512],
                    start=start, stop=stop,
                    tile_position=(0, bp) if COLPACK else None,
                    skip_group_check=COLPACK,
                )

        # ---- evacuate the two accumulator slices and DMA out ----
        sbf = const.tile([128, QL], FP32)
        nc.vector.tensor_copy(out=sbf[0:CA, :], in_=outS_ps[0:CA, :])
        nc.sync.dma_start(out=d_out[0:CA, :], in_=sbf[0:CA, :])
        if COLPACK:
            nc.vector.tensor_copy(
                out=sbf[64:64 + CA, :], in_=outS_ps[64:64 + CA, :]
            )
            nc.sync.dma_start(out=d_out[CA:2 * CA, :], in_=sbf[64:64 + CA, :])


def build_program(niters=1):
    nc = bacc.Bacc(
        "TRN2", target_bir_lowering=False, debug=False, num_devices=NCORES
    )
    d_xtT = nc.dram_tensor("xtT", [CQ, QL], FP32, kind="ExternalInput").ap()
    d_ytT = nc.dram_tensor("ytT", [CK, NV], FP32, kind="ExternalInput").ap()
    d_ytaug = nc.dram_tensor(
        "ytaug", [128, NKB * CA], ATTDT, kind="ExternalInput"
    ).ap()
    d_wpT = nc.dram_tensor("wpT", [CQ, F], FP32, kind="ExternalInput").ap()
    d_wgT = nc.dram_tensor("wgT", [CK, F], FP32, kind="ExternalInput").ap()
    d_out = nc.dram_tensor(
        "outS2", [2 * CA, QL], FP32, kind="ExternalOutput"
    ).ap()

    with tile.TileContext(nc) as tc:
        _body(tc, d_xtT, d_ytT, d_ytaug, d_wpT, d_wgT, d_out, niters=niters)
    nc.compile()
    return nc


def get_program():
    global _PROG
    if _PROG is None:
        _PROG = build_program()
    return _PROG


def make_in_maps(x, y, masks):
    """Host-side sharding: gather the valid tokens, lay them out for the
    device, and split the queries across the 8 cores."""
    x = np.ascontiguousarray(np.asarray(x, dtype=np.float32))
    y = np.ascontiguousarray(np.asarray(y, dtype=np.float32))
    mflat = np.asarray(masks).reshape(-1)
    idx = np.flatnonzero(mflat > 0)
    assert idx.size == NV, f"expected {NV} valid tokens, got {idx.size}"

    xt = x.transpose(0, 2, 3, 1).reshape(-1, CQ)[idx]            # [NV, CQ]
    yt = y.transpose(0, 2, 3, 1).reshape(-1, CK)[idx]            # [NV, CK]
    ytT = np.ascontiguousarray(yt.T)                             # [CK, NV]
    ytaug = np.concatenate(
        [yt, np.ones((NV, 1), np.float32)], axis=1
    )                                                            # [NV, CA]
    import ml_dtypes
    ytaug_dev = np.ascontiguousarray(
        ytaug.reshape(NKB, 128, CA).transpose(1, 0, 2).reshape(128, NKB * CA)
    ).astype(ml_dtypes.bfloat16)
    in_maps = []
    for c in range(NCORES):
        xtT_c = np.ascontiguousarray(xt[c * QL:(c + 1) * QL].T)  # [CQ, QL]
        in_maps.append({
            "xtT": xtT_c,
            "ytT": ytT,
            "ytaug": ytaug_dev,
            "wpT": None,  # filled by caller (weights)
            "wgT": None,
        })
    return idx, in_maps


def kernel(x, y, masks, Wp, Wt, Wg, Wz, gn_w, gn_b, trace=False):
    global LAST_RESULTS
    x = np.ascontiguousarray(np.asarray(x, dtype=np.float32))
    Wp = np.asarray(Wp, dtype=np.float32)
    Wt = np.asarray(Wt, dtype=np.float32)
    Wg = np.asarray(Wg, dtype=np.float32)
    Wz = np.asarray(Wz, dtype=np.float32)
    gn_w = np.asarray(gn_w, dtype=np.float32)
    gn_b = np.asarray(gn_b, dtype=np.float32)

    idx, in_maps = make_in_maps(x, y, masks)
    wpT = np.ascontiguousarray(Wp.T)                             # [CQ, F]
    wgT = np.ascontiguousarray(Wg.T)                             # [CK, F]
    for m in in_maps:
        m["wpT"] = wpT
        m["wgT"] = wgT

    nc = get_program()
    try:
        res = bass_utils.run_bass_kernel_spmd(
            nc, in_maps, core_ids=list(range(NCORES)), trace=trace
        )
    except ModuleNotFoundError:
        # NTFF profiling hook unavailable in this environment
        res = bass_utils.run_bass_kernel_spmd(
            nc, in_maps, core_ids=list(range(NCORES)), trace=False
        )
    LAST_RESULTS = res

    # ---- host-side unshard: normalize, fold value/output projection,
    # global group norm, scatter, residual ----
    Wzt = Wz.astype(np.float64) @ Wt.astype(np.float64)          # [CQ, CK]
    z = np.empty((NV, CQ), np.float32)
    for c in range(NCORES):
        o = res.results[c]["outS2"].astype(np.float64)           # [2*CA, QL]
        outS = o[0:CA] + o[CA:2 * CA] if COLPACK else o[0:CA]    # [CA, QL]
        att = outS[0:CK] / outS[CK:CK + 1]                       # [CK, QL]
        z[c * QL:(c + 1) * QL] = (Wzt @ att).T.astype(np.float32)
    zd = z.astype(np.float64)
    mu = zd.mean()
    var = zd.var()
    zn = ((zd - mu) / np.sqrt(var + EPS)).astype(np.float32)
    zn = zn * gn_w[None, :] + gn_b[None, :]
    out_tok = np.zeros((B * H * W, CQ), np.float32)
    out_tok[idx] = zn
    out = out_tok.reshape(B, H * W, CQ).transpose(0, 2, 1).reshape(B, CQ, H, W)
    return out + x



# revision 4
# speedup vs baseline: 4.1044x; 4.1044x over previous
"""Sparse cross-attention kernel for Trainium2 (8 NeuronCores).

Sharding: valid tokens (rows of the NxN attention) are sharded across the 8
cores -- each core holds 1024 queries and the full gathered key/value set
(8192 tokens), computes its energy rows + softmax + output rows.

The wall-clock of a call is dominated by the axon tunnel (~80ms round trip,
~20ms/MB upload), so the I/O format is optimized for the wire:
  * ONE packed bf16 input tensor per core (each extra jit array argument
    costs ~55ms; fp32 would double the bytes). Layout per core, [164, 1024]:
      rows   0:32   xtT   [32, 1024]  queries, channel-major
      rows  32:160  ytT   [16, 8192]  keys/values, channel-major (flattened)
      rows 160:162  wpT   [32, 64]    flattened
      row  162      wgT   [16, 64]    flattened
      row  163      eye16 [16, 16]    flattened (transpose identity)
  * ONE fp32 output tensor [17, 1024] per core: 16 attention-output
    numerator rows + the softmax denominator row; division and the tiny
    (Wz @ Wt) projection happen on the host.
  * the PJRT output-donation buffer is kept device-resident across calls
    (the kernel overwrites every output element, so its contents are
    irrelevant) -- zero tunnel bytes for it.

Device layout trick (from the fp32 baseline): energy is computed TRANSPOSED
(eT[key, query], keys on partitions) so that
  * the exp for softmax is a single ScalarE pass PSUM->SBUF,
  * the attention matmul consumes exp(eT) directly as the moving operand with
    token-major value tiles as stationary weights,
  * a ones-lane in the value tiles makes the softmax denominator fall out of
    the same accumulation for free.
The token-major value tiles [128, 17] per key block are derived on device
from the channel-major ytT via 64 PE transposes (so yt crosses the wire
exactly once, in one layout).
"""

import sys

import numpy as np

sys.path.insert(0, "/opt/trn_rl_repo")

import ml_dtypes  # noqa: E402

import concourse.bacc as bacc  # noqa: E402
import concourse.tile as tile  # noqa: E402
from concourse import mybir  # noqa: E402
from concourse.bass import AP  # noqa: E402

# problem constants (hardcoded per contract)
B, CQ, CK, F, H, W = 2, 32, 16, 64, 128, 128
NV = 8192               # number of valid (mask > 0) tokens
NCORES = 8
QL = NV // NCORES       # queries per core
KB = 128                # key block (partition dim of eT tiles)
NKB = NV // KB          # 64 key blocks
CA = CK + 1             # value channels + ones lane
NQMM = QL // 512        # moving-dim chunks per matmul (fp32 max free 512)
EPS = 1e-5

PKR = 164               # packed input rows per core (see module docstring)

FP32 = mybir.dt.float32
FP32R = mybir.dt.float32r
BF16 = mybir.dt.bfloat16

BF = ml_dtypes.bfloat16

_EXEC = None            # (sharded_jit_fn, donation_buffer)
LAST_RESULTS = None     # kept for the test harness (always None here)


def _body(tc, d_pk_h, d_out):
    nc = tc.nc
    from contextlib import ExitStack

    def pk_ap(offset, ap):
        return AP(d_pk_h, offset, ap)

    with ExitStack() as ctx:
        const = ctx.enter_context(tc.tile_pool(name="const", bufs=1))
        xpool = ctx.enter_context(tc.tile_pool(name="xp", bufs=8))
        epool = ctx.enter_context(tc.tile_pool(name="ep", bufs=2, space="PSUM"))
        apool = ctx.enter_context(tc.tile_pool(name="acc", bufs=1, space="PSUM"))
        tpool = ctx.enter_context(tc.tile_pool(name="tp", bufs=2, space="PSUM"))

        # ---- input DMAs (all from the one packed tensor) ----
        wp_s = const.tile([CQ, F], BF16)
        nc.sync.dma_start(out=wp_s[:], in_=pk_ap(160 * 1024, [[F, CQ], [1, F]]))
        wg_s = const.tile([CK, F], BF16)
        nc.sync.dma_start(out=wg_s[:], in_=pk_ap(162 * 1024, [[F, CK], [1, F]]))
        id_s = const.tile([CK, CK], BF16)
        nc.sync.dma_start(out=id_s[:], in_=pk_ap(163 * 1024, [[CK, CK], [1, CK]]))
        xtT_s = const.tile([CQ, QL], BF16)
        nc.sync.dma_start(out=xtT_s[:], in_=pk_ap(0, [[1024, CQ], [1, 1024]]))
        # chunk the key DMA so dependent work can start on the first slice
        ytT_s = const.tile([CK, NV], BF16)
        for c in range(4):
            w = NV // 4
            nc.sync.dma_start(
                out=ytT_s[:, c * w:(c + 1) * w],
                in_=pk_ap(32 * 1024 + c * w, [[NV, CK], [1, w]]),
            )

        # ---- query projection: pT[f, q] = WpT.T @ xtT ----
        p_ps = epool.tile([F, QL], FP32, tag="et")
        for i in range(NQMM):
            nc.tensor.matmul(
                out=p_ps[:, i * 512:(i + 1) * 512],
                lhsT=wp_s[:],
                rhs=xtT_s[:, i * 512:(i + 1) * 512],
                start=True, stop=True,
            )
        pT_s = const.tile([F, QL], FP32R)
        nc.vector.tensor_copy(out=pT_s[:], in_=p_ps[:])

        # ---- key projection: gT[f, k] = WgT.T @ ytT ----
        gT_s = const.tile([F, NV], FP32R)
        for c in range(16):
            g_ps = epool.tile([F, 512], FP32, tag="et")
            nc.tensor.matmul(
                out=g_ps[:],
                lhsT=wg_s[:],
                rhs=ytT_s[:, c * 512:(c + 1) * 512],
                start=True, stop=True,
            )
            nc.vector.tensor_copy(out=gT_s[:, c * 512:(c + 1) * 512], in_=g_ps[:])

        # ---- token-major value tiles [128, 17] per key block, derived from
        # ytT via PE transposes; lane 16 of each block stays at the memset
        # 1.0 and becomes the softmax-denominator accumulator ----
        vt_s = const.tile([KB, NKB * CA], BF16)
        nc.vector.memset(vt_s[:], 1.0)
        for j in range(NKB):
            tp = tpool.tile([KB, CK], BF16, tag="tp")
            nc.tensor.transpose(
                tp[:], ytT_s[:, j * KB:(j + 1) * KB], id_s[:]
            )
            nc.vector.tensor_copy(
                out=vt_s[:, j * CA:j * CA + CK], in_=tp[:]
            )

        # ---- attention accumulator: vt.T @ exp(eT), [17, QL] per column
        # group. The two groups share PSUM banks at disjoint partition
        # ranges (0:17 and 64:81) -- the standard col-tiling layout; PSUM
        # has_written bits are per-element so the interleaved accumulation
        # groups don't interact (the sim's coarse zero-region tracker can't
        # see that, hence skip_group_check on the matmuls). ----
        outS_ps = apool.tile([128, QL], FP32)

        for j in range(NKB):
            # energy block (transposed): eT[k, q] = g_k . p_q
            e_ps = epool.tile([128, QL], FP32, tag="et")
            lhs_g = gT_s[:, j * KB:(j + 1) * KB]
            for i in range(NQMM):
                nc.tensor.matmul(
                    out=e_ps[:, i * 512:(i + 1) * 512],
                    lhsT=lhs_g,
                    rhs=pT_s[:, i * 512:(i + 1) * 512],
                    start=True, stop=True,
                )
            # softmax numerator: exp straight out of PSUM into SBUF.
            # No max subtraction: energies are O(+-50), well inside fp32 exp
            # range, and the reference's max-shift cancels mathematically.
            x_s = xpool.tile([128, QL], BF16, tag="xp")
            nc.scalar.activation(
                out=x_s[:], in_=e_ps[:], func=mybir.ActivationFunctionType.Exp
            )
            # attention matmul, accumulated over key blocks; even/odd blocks
            # go to different PE column groups (disjoint PSUM partitions) so
            # consecutive blocks run concurrently on the array
            grp = j % 2
            bp = 64 * grp
            lhs_t = vt_s[:, j * CA:(j + 1) * CA]
            start = j == grp
            stop = j == (NKB - 2 + grp)
            for i in range(NQMM):
                nc.tensor.matmul(
                    out=outS_ps[bp:bp + CA, i * 512:(i + 1) * 512],
                    lhsT=lhs_t,
                    rhs=x_s[:, i * 512:(i + 1) * 512],
                    start=start, stop=stop,
                    tile_position=(0, bp),
                    skip_group_check=True,
                )

        # ---- sum the two accumulator groups and DMA out (only one PSUM
        # operand allowed per DVE instruction, so evacuate one group first)
        out_s = const.tile([CA, QL], FP32)
        nc.vector.tensor_copy(out=out_s[:], in_=outS_ps[0:CA, :])
        nc.vector.tensor_tensor(
            out_s[:], out_s[:], outS_ps[64:64 + CA, :],
            mybir.AluOpType.add,
        )
        nc.sync.dma_start(out=d_out[:], in_=out_s[:])


def build_program():
    nc = bacc.Bacc(
        "TRN2", target_bir_lowering=False, debug=False, num_devices=NCORES
    )
    d_pk_h = nc.dram_tensor("pk", [PKR, 1024], BF16, kind="ExternalInput")
    d_out = nc.dram_tensor("outk", [CA, QL], FP32, kind="ExternalOutput").ap()

    with tile.TileContext(nc) as tc:
        _body(tc, d_pk_h, d_out)
    nc.compile()
    return nc


def _build_exec():
    """Compile the program and build a cached jitted SPMD callable.

    Mirrors concourse.bass_utils.run_bass_kernel_spmd's axon path
    (bass2jax.run_bass_via_pjrt), but hoists everything reusable out of the
    per-call path: the jitted executable, and the device-resident donation
    buffer for the output (the kernel writes every output element, so the
    buffer's contents don't matter and it never has to cross the tunnel).
    """
    import jax
    import jax.numpy as jnp
    from jax.sharding import Mesh, NamedSharding, PartitionSpec
    from jax.experimental.shard_map import shard_map
    from concourse import bass2jax
    from concourse.bass2jax import _bass_exec_p, install_neuronx_cc_hook

    nc = build_program()
    install_neuronx_cc_hook()

    # derive parameter order exactly the way run_bass_via_pjrt does
    partition_name = (
        nc.partition_id_tensor.name if nc.partition_id_tensor else None
    )
    in_names, out_names, out_avals = [], [], []
    for alloc in nc.m.functions[0].allocations:
        if not isinstance(alloc, mybir.MemoryLocationSet):
            continue
        name = alloc.memorylocations[0].name
        if alloc.kind == "ExternalInput":
            if name != partition_name:
                in_names.append(name)
        elif alloc.kind == "ExternalOutput":
            out_names.append(name)
            out_avals.append(
                jax.core.ShapedArray(
                    tuple(alloc.tensor_shape), mybir.dt.np(alloc.dtype)
                )
            )
    assert in_names == ["pk"] and out_names == ["outk"], (in_names, out_names)
    all_in_names = in_names + out_names
    if partition_name is not None:
        all_in_names.append(partition_name)

    def _per_core(pk, ob):
        operands = [pk, ob]
        if partition_name is not None:
            operands.append(bass2jax.partition_id_tensor())
        outs = _bass_exec_p.bind(
            *operands,
            out_avals=tuple(out_avals),
            in_names=tuple(all_in_names),
            out_names=tuple(out_names),
            lowering_input_output_aliases=(),
            sim_require_finite=True,
            sim_require_nnan=True,
            nc=nc,
        )
        return outs[0]

    devices = jax.devices()[:NCORES]
    mesh = Mesh(np.asarray(devices), ("core",))
    spec = PartitionSpec("core")
    sharded = jax.jit(
        shard_map(
            _per_core, mesh=mesh, in_specs=(spec, spec), out_specs=spec,
            check_rep=False,
        ),
        keep_unused=True,
    )
    ob = jax.device_put(
        np.zeros((NCORES * CA, QL), np.float32), NamedSharding(mesh, spec)
    )
    return sharded, ob


def _get_exec():
    global _EXEC
    if _EXEC is None:
        _EXEC = _build_exec()
    return _EXEC


_PKBUF = None


def _pack_inputs(x, y, masks, Wp, Wg):
    """Gather the valid tokens and lay them out in the packed wire format."""
    global _PKBUF
    if _PKBUF is None:
        _PKBUF = np.zeros((NCORES * PKR, 1024), BF)
    PK = _PKBUF

    mflat = np.asarray(masks).reshape(-1)
    idx = np.flatnonzero(mflat > 0)
    assert idx.size == NV, f"expected {NV} valid tokens, got {idx.size}"
    b_idx = idx // (H * W)
    s_idx = idx % (H * W)

    xt = x.reshape(B, CQ, H * W)[b_idx, :, s_idx]        # [NV, CQ] fp32
    yt = y.reshape(B, CK, H * W)[b_idx, :, s_idx]        # [NV, CK] fp32
    xt16 = xt.astype(BF)
    ytT16 = np.ascontiguousarray(yt.T).astype(BF)        # [CK, NV]
    ytT_rows = ytT16.reshape(KB, 1024)
    wp_rows = np.ascontiguousarray(Wp.T).astype(BF).reshape(2, 1024)
    wg_row = np.ascontiguousarray(Wg.T).astype(BF).reshape(1024)
    eye_row = np.eye(CK, dtype=BF).reshape(-1)

    for c in range(NCORES):
        base = c * PKR
        PK[base:base + CQ] = xt16[c * QL:(c + 1) * QL].T
        PK[base + CQ:base + 160] = ytT_rows
        PK[base + 160:base + 162] = wp_rows
        PK[base + 162] = wg_row
        PK[base + 163, :CK * CK] = eye_row
    return idx, b_idx, s_idx, PK


def kernel(x, y, masks, Wp, Wt, Wg, Wz, gn_w, gn_b, trace=False):
    x = np.ascontiguousarray(np.asarray(x, dtype=np.float32))
    y = np.ascontiguousarray(np.asarray(y, dtype=np.float32))
    Wp = np.asarray(Wp, dtype=np.float32)
    Wt = np.asarray(Wt, dtype=np.float32)
    Wg = np.asarray(Wg, dtype=np.float32)
    Wz = np.asarray(Wz, dtype=np.float32)
    gn_w = np.asarray(gn_w, dtype=np.float32)
    gn_b = np.asarray(gn_b, dtype=np.float32)

    sharded, ob = _get_exec()
    idx, b_idx, s_idx, PK = _pack_inputs(x, y, masks, Wp, Wg)
    out = np.asarray(sharded(PK, ob))                    # [8*17, QL] fp32

    # ---- host-side unshard: softmax division, fold value/output projection,
    # global group norm, scatter, residual ----
    o = out.reshape(NCORES, CA, QL).astype(np.float64)
    att = o[:, :CK, :] / o[:, CK:CK + 1, :]              # [8, CK, QL]
    Wzt = Wz.astype(np.float64) @ Wt.astype(np.float64)  # [CQ, CK]
    z = np.einsum("qc,ncl->nql", Wzt, att, optimize=True)
    z = z.transpose(0, 2, 1).reshape(NV, CQ)             # [NV, CQ]
    mu = z.mean()
    var = z.var()
    zn = ((z - mu) / np.sqrt(var + EPS)).astype(np.float32)
    zn = zn * gn_w[None, :] + gn_b[None, :]
    res = x.copy()
    rv = res.reshape(B, CQ, H * W)
    rv[b_idx, :, s_idx] = rv[b_idx, :, s_idx] + zn
    return res


# revision 9
# speedup vs baseline: 5.4363x; 1.3245x over previous
"""Sparse cross-attention kernel for Trainium2 (8 NeuronCores).

Sharding: valid tokens (rows of the NxN attention) are sharded across the 8
cores -- each core holds 1024 queries and the full gathered key/value set
(8192 tokens), computes its energy rows + softmax + output rows.

The wall-clock of a call is dominated by the axon tunnel (~80ms round trip,
~20ms/MB upload), so the I/O format is optimized for the wire:
  * ONE packed bf16 input tensor per core (each extra jit array argument
    costs ~55ms; fp32 would double the bytes). Layout per core, [164, 1024]:
      rows   0:32   xtT   [32, 1024]  queries, channel-major
      rows  32:160  ytT   [16, 8192]  keys/values, channel-major (flattened)
      rows 160:162  wpT   [32, 64]    flattened
      row  162      wgT   [16, 64]    flattened
      row  163      eye16 [16, 16]    flattened (transpose identity)
  * ONE fp32 output tensor [17, 1024] per core: 16 attention-output
    numerator rows + the softmax denominator row; division and the tiny
    (Wz @ Wt) projection happen on the host.
  * the PJRT output-donation buffer is kept device-resident across calls
    (the kernel overwrites every output element, so its contents are
    irrelevant) -- zero tunnel bytes for it.

Device layout trick (from the fp32 baseline): energy is computed TRANSPOSED
(eT[key, query], keys on partitions) so that
  * the exp for softmax is a single ScalarE pass PSUM->SBUF,
  * the attention matmul consumes exp(eT) directly as the moving operand with
    token-major value tiles as stationary weights,
  * a ones-lane in the value tiles makes the softmax denominator fall out of
    the same accumulation for free.
The token-major value tiles [128, 17] per key block are derived on device
from the channel-major ytT via 64 PE transposes (so yt crosses the wire
exactly once, in one layout).
"""

import sys

import numpy as np

sys.path.insert(0, "/opt/trn_rl_repo")

import ml_dtypes  # noqa: E402

import concourse.bacc as bacc  # noqa: E402
import concourse.tile as tile  # noqa: E402
from concourse import mybir  # noqa: E402
from concourse.bass import AP  # noqa: E402

# problem constants (hardcoded per contract)
B, CQ, CK, F, H, W = 2, 32, 16, 64, 128, 128
NV = 8192               # number of valid (mask > 0) tokens
NCORES = 8
QL = NV // NCORES       # queries per core
KB = 128                # key block (partition dim of eT tiles)
NKB = NV // KB          # 64 key blocks
CA = CK + 1             # value channels + ones lane
NQMM = QL // 512        # moving-dim chunks per matmul (fp32 max free 512)
EPS = 1e-5

USE_CC = True           # shard yt across cores + on-device AllGather
PKR = 52 if USE_CC else 164  # packed input rows per core (see module docstring)
# USE_CC=True layout per core, [52, 1024] bf16:
#   rows  0:32   xtT        [32, 1024]  this core's queries, channel-major
#   rows 32:48   ytT shard  [16, 1024]  this core's 1024 tokens, channel-major
#   rows 48:50   wpT        [32, 64]    flattened
#   row  50      wgT        [16, 64]    flattened
#   row  51      eye16      [16, 16]    flattened
YTOFF = (CQ if USE_CC else 32) * 1024            # ytT region offset
WOFF = (48 if USE_CC else 160) * 1024            # weights region offset

FP32 = mybir.dt.float32
FP32R = mybir.dt.float32r
BF16 = mybir.dt.bfloat16

BF = ml_dtypes.bfloat16

_EXEC = None            # (sharded_jit_fn, donation_buffer)
LAST_RESULTS = None     # kept for the test harness (always None here)


def _body(tc, d_pk_h, d_cc_in_h, d_cc_out_h, d_out):
    nc = tc.nc
    from contextlib import ExitStack

    def pk_ap(offset, ap):
        return AP(d_pk_h, offset, ap)

    with ExitStack() as ctx:
        const = ctx.enter_context(tc.tile_pool(name="const", bufs=1))
        xpool = ctx.enter_context(tc.tile_pool(name="xp", bufs=8))
        epool = ctx.enter_context(tc.tile_pool(name="ep", bufs=2, space="PSUM"))
        apool = ctx.enter_context(tc.tile_pool(name="acc", bufs=1, space="PSUM"))
        tpool = ctx.enter_context(tc.tile_pool(name="tp", bufs=2, space="PSUM"))

        # ---- input DMAs (all from the one packed tensor) ----
        wp_s = const.tile([CQ, F], BF16)
        nc.sync.dma_start(out=wp_s[:], in_=pk_ap(WOFF, [[F, CQ], [1, F]]))
        wg_s = const.tile([CK, F], BF16)
        nc.sync.dma_start(
            out=wg_s[:], in_=pk_ap(WOFF + 2 * 1024, [[F, CK], [1, F]])
        )
        id_s = const.tile([CK, CK], BF16)
        nc.sync.dma_start(
            out=id_s[:], in_=pk_ap(WOFF + 3 * 1024, [[CK, CK], [1, CK]])
        )
        xtT_s = const.tile([CQ, QL], BF16)
        nc.sync.dma_start(out=xtT_s[:], in_=pk_ap(0, [[1024, CQ], [1, 1024]]))
        ytT_s = const.tile([CK, NV], BF16)
        if USE_CC:
            # each core contributes its own 1024 tokens; AllGather the
            # full 8192-token channel-major key/value set over NeuronLink
            # (32KB/core on the wire instead of 8x256KB from the host).
            # Collectives need non-I/O HBM bounce tensors on both sides.
            nc.sync.dma_start(out=d_cc_in_h.ap(), in_=pk_ap(
                YTOFF, [[1024, CK], [1, 1024]]
            ))
            nc.gpsimd.collective_compute(
                "AllGather",
                mybir.AluOpType.bypass,
                replica_groups=[list(range(NCORES))],
                ins=[d_cc_in_h.ap()],
                outs=[d_cc_out_h.ap()],
            )
            # gathered layout is [core r][channel c][local token kl];
            # pull it into [c, r*1024 + kl]
            nc.sync.dma_start(
                out=ytT_s[:],
                in_=AP(d_cc_out_h, 0,
                       [[1024, CK], [CK * 1024, NCORES], [1, 1024]]),
            )
        else:
            # chunk the key DMA so dependent work can start early
            for c in range(4):
                w = NV // 4
                nc.sync.dma_start(
                    out=ytT_s[:, c * w:(c + 1) * w],
                    in_=pk_ap(YTOFF + c * w, [[NV, CK], [1, w]]),
                )

        # ---- query projection: pT[f, q] = WpT.T @ xtT ----
        p_ps = epool.tile([F, QL], FP32, tag="et")
        for i in range(NQMM):
            nc.tensor.matmul(
                out=p_ps[:, i * 512:(i + 1) * 512],
                lhsT=wp_s[:],
                rhs=xtT_s[:, i * 512:(i + 1) * 512],
                start=True, stop=True,
            )
        pT_s = const.tile([F, QL], FP32R)
        nc.vector.tensor_copy(out=pT_s[:], in_=p_ps[:])

        # ---- key projection: gT[f, k] = WgT.T @ ytT ----
        gT_s = const.tile([F, NV], FP32R)
        for c in range(16):
            g_ps = epool.tile([F, 512], FP32, tag="et")
            nc.tensor.matmul(
                out=g_ps[:],
                lhsT=wg_s[:],
                rhs=ytT_s[:, c * 512:(c + 1) * 512],
                start=True, stop=True,
            )
            nc.vector.tensor_copy(out=gT_s[:, c * 512:(c + 1) * 512], in_=g_ps[:])

        # ---- token-major value tiles [128, 17] per key block, derived from
        # ytT via PE transposes; lane 16 of each block stays at the memset
        # 1.0 and becomes the softmax-denominator accumulator ----
        vt_s = const.tile([KB, NKB * CA], BF16)
        nc.vector.memset(vt_s[:], 1.0)
        for j in range(NKB):
            tp = tpool.tile([KB, CK], BF16, tag="tp")
            nc.tensor.transpose(
                tp[:], ytT_s[:, j * KB:(j + 1) * KB], id_s[:]
            )
            nc.vector.tensor_copy(
                out=vt_s[:, j * CA:j * CA + CK], in_=tp[:]
            )

        # ---- attention accumulator: vt.T @ exp(eT), [17, QL] per column
        # group. The two groups share PSUM banks at disjoint partition
        # ranges (0:17 and 64:81) -- the standard col-tiling layout; PSUM
        # has_written bits are per-element so the interleaved accumulation
        # groups don't interact (the sim's coarse zero-region tracker can't
        # see that, hence skip_group_check on the matmuls). ----
        outS_ps = apool.tile([128, QL], FP32)

        for j in range(NKB):
            # energy block (transposed): eT[k, q] = g_k . p_q
            e_ps = epool.tile([128, QL], FP32, tag="et")
            lhs_g = gT_s[:, j * KB:(j + 1) * KB]
            for i in range(NQMM):
                nc.tensor.matmul(
                    out=e_ps[:, i * 512:(i + 1) * 512],
                    lhsT=lhs_g,
                    rhs=pT_s[:, i * 512:(i + 1) * 512],
                    start=True, stop=True,
                )
            # softmax numerator: exp straight out of PSUM into SBUF.
            # No max subtraction: energies are O(+-50), well inside fp32 exp
            # range, and the reference's max-shift cancels mathematically.
            x_s = xpool.tile([128, QL], BF16, tag="xp")
            nc.scalar.activation(
                out=x_s[:], in_=e_ps[:], func=mybir.ActivationFunctionType.Exp
            )
            # attention matmul, accumulated over key blocks; even/odd blocks
            # go to different PE column groups (disjoint PSUM partitions) so
            # consecutive blocks run concurrently on the array
            grp = j % 2
            bp = 64 * grp
            lhs_t = vt_s[:, j * CA:(j + 1) * CA]
            start = j == grp
            stop = j == (NKB - 2 + grp)
            for i in range(NQMM):
                nc.tensor.matmul(
                    out=outS_ps[bp:bp + CA, i * 512:(i + 1) * 512],
                    lhsT=lhs_t,
                    rhs=x_s[:, i * 512:(i + 1) * 512],
                    start=start, stop=stop,
                    tile_position=(0, bp),
                    skip_group_check=True,
                )

        # ---- sum the two accumulator groups and DMA out (only one PSUM
        # operand allowed per DVE instruction, so evacuate one group first)
        out_s = const.tile([CA, QL], FP32)
        nc.vector.tensor_copy(out=out_s[:], in_=outS_ps[0:CA, :])
        nc.vector.tensor_tensor(
            out_s[:], out_s[:], outS_ps[64:64 + CA, :],
            mybir.AluOpType.add,
        )
        nc.sync.dma_start(out=d_out[:], in_=out_s[:])


def build_program():
    nc = bacc.Bacc(
        "TRN2", target_bir_lowering=False, debug=False, num_devices=NCORES
    )
    d_pk_h = nc.dram_tensor("pk", [PKR, 1024], BF16, kind="ExternalInput")
    d_out = nc.dram_tensor("outk", [CA, QL], FP32, kind="ExternalOutput").ap()
    d_cc_in_h = d_cc_out_h = None
    if USE_CC:
        d_cc_in_h = nc.dram_tensor("cc_in", [CK, QL], BF16)
        d_cc_out_h = nc.dram_tensor("cc_out", [NCORES, CK, QL], BF16)

    with tile.TileContext(nc) as tc:
        _body(tc, d_pk_h, d_cc_in_h, d_cc_out_h, d_out)
    nc.compile()
    return nc


def _build_exec():
    """Compile the program and build a cached jitted SPMD callable.

    Mirrors concourse.bass_utils.run_bass_kernel_spmd's axon path
    (bass2jax.run_bass_via_pjrt), but hoists everything reusable out of the
    per-call path: the jitted executable, and the device-resident donation
    buffer for the output (the kernel writes every output element, so the
    buffer's contents don't matter and it never has to cross the tunnel).
    """
    import jax
    import jax.numpy as jnp
    from jax.sharding import Mesh, NamedSharding, PartitionSpec
    from jax.experimental.shard_map import shard_map
    from concourse import bass2jax
    from concourse.bass2jax import _bass_exec_p, install_neuronx_cc_hook

    nc = build_program()
    install_neuronx_cc_hook()

    # derive parameter order exactly the way run_bass_via_pjrt does
    partition_name = (
        nc.partition_id_tensor.name if nc.partition_id_tensor else None
    )
    in_names, out_names, out_avals = [], [], []
    for alloc in nc.m.functions[0].allocations:
        if not isinstance(alloc, mybir.MemoryLocationSet):
            continue
        name = alloc.memorylocations[0].name
        if alloc.kind == "ExternalInput":
            if name != partition_name:
                in_names.append(name)
        elif alloc.kind == "ExternalOutput":
            out_names.append(name)
            out_avals.append(
                jax.core.ShapedArray(
                    tuple(alloc.tensor_shape), mybir.dt.np(alloc.dtype)
                )
            )
    assert in_names == ["pk"] and out_names == ["outk"], (in_names, out_names)
    all_in_names = in_names + out_names
    if partition_name is not None:
        all_in_names.append(partition_name)

    def _per_core(pk, ob):
        operands = [pk, ob]
        if partition_name is not None:
            operands.append(bass2jax.partition_id_tensor())
        outs = _bass_exec_p.bind(
            *operands,
            out_avals=tuple(out_avals),
            in_names=tuple(all_in_names),
            out_names=tuple(out_names),
            lowering_input_output_aliases=(),
            sim_require_finite=True,
            sim_require_nnan=True,
            nc=nc,
        )
        return outs[0]

    devices = jax.devices()[:NCORES]
    mesh = Mesh(np.asarray(devices), ("core",))
    spec = PartitionSpec("core")
    sharded = jax.jit(
        shard_map(
            _per_core, mesh=mesh, in_specs=(spec, spec), out_specs=spec,
            check_rep=False,
        ),
        keep_unused=True,
    )
    ob = jax.device_put(
        np.zeros((NCORES * CA, QL), np.float32), NamedSharding(mesh, spec)
    )
    return sharded, ob


def _get_exec():
    global _EXEC
    if _EXEC is None:
        _EXEC = _build_exec()
    return _EXEC


_PKBUF = None


def _pack_inputs(x, y, masks, Wp, Wg):
    """Gather the valid tokens and lay them out in the packed wire format."""
    global _PKBUF
    if _PKBUF is None:
        _PKBUF = np.zeros((NCORES * PKR, 1024), BF)
    PK = _PKBUF

    mflat = np.asarray(masks).reshape(-1)
    idx = np.flatnonzero(mflat > 0)
    assert idx.size == NV, f"expected {NV} valid tokens, got {idx.size}"
    b_idx = idx // (H * W)
    s_idx = idx % (H * W)

    xt = x.reshape(B, CQ, H * W)[b_idx, :, s_idx]        # [NV, CQ] fp32
    yt = y.reshape(B, CK, H * W)[b_idx, :, s_idx]        # [NV, CK] fp32
    xt16 = xt.astype(BF)
    ytT16 = np.ascontiguousarray(yt.T).astype(BF)        # [CK, NV]
    wp_rows = np.ascontiguousarray(Wp.T).astype(BF).reshape(2, 1024)
    wg_row = np.ascontiguousarray(Wg.T).astype(BF).reshape(1024)
    eye_row = np.eye(CK, dtype=BF).reshape(-1)

    wrow = WOFF // 1024
    for c in range(NCORES):
        base = c * PKR
        PK[base:base + CQ] = xt16[c * QL:(c + 1) * QL].T
        if USE_CC:
            PK[base + CQ:base + CQ + CK] = ytT16[:, c * QL:(c + 1) * QL]
        else:
            PK[base + CQ:base + CQ + KB] = ytT16.reshape(KB, 1024)
        PK[base + wrow:base + wrow + 2] = wp_rows
        PK[base + wrow + 2] = wg_row
        PK[base + wrow + 3, :CK * CK] = eye_row
    return idx, b_idx, s_idx, PK


def kernel(x, y, masks, Wp, Wt, Wg, Wz, gn_w, gn_b, trace=False):
    x = np.ascontiguousarray(np.asarray(x, dtype=np.float32))
    y = np.ascontiguousarray(np.asarray(y, dtype=np.float32))
    Wp = np.asarray(Wp, dtype=np.float32)
    Wt = np.asarray(Wt, dtype=np.float32)
    Wg = np.asarray(Wg, dtype=np.float32)
    Wz = np.asarray(Wz, dtype=np.float32)
    gn_w = np.asarray(gn_w, dtype=np.float32)
    gn_b = np.asarray(gn_b, dtype=np.float32)

    sharded, ob = _get_exec()
    idx, b_idx, s_idx, PK = _pack_inputs(x, y, masks, Wp, Wg)
    out = np.asarray(sharded(PK, ob))                    # [8*17, QL] fp32

    # ---- host-side unshard: softmax division, fold value/output projection,
    # global group norm, scatter, residual ----
    o = out.reshape(NCORES, CA, QL).astype(np.float64)
    att = o[:, :CK, :] / o[:, CK:CK + 1, :]              # [8, CK, QL]
    Wzt = Wz.astype(np.float64) @ Wt.astype(np.float64)  # [CQ, CK]
    z = np.einsum("qc,ncl->nql", Wzt, att, optimize=True)
    z = z.transpose(0, 2, 1).reshape(NV, CQ)             # [NV, CQ]
    mu = z.mean()
    var = z.var()
    zn = ((z - mu) / np.sqrt(var + EPS)).astype(np.float32)
    zn = zn * gn_w[None, :] + gn_b[None, :]
    res = x.copy()
    rv = res.reshape(B, CQ, H * W)
    rv[b_idx, :, s_idx] = rv[b_idx, :, s_idx] + zn
    return res
